# revision 9
# baseline (speedup 1.0000x reference)
"""Trainium2 Bass kernel for AdaptiveCLPLLoss.

Reference computation (B=512, C=100000, HEAD=2000, K=10, S=100):
    logits  [B, C] f32, candidates [B, K] i64, sampled_idx [S] i64
    y_mask  = binarized scatter of valid candidates            [B, C]
    term1   = softplus(-avg_cand)     avg over distinct candidate logits
    term2   = sum over head cols of softplus(logits) * (1 - y_mask)
    term3   = sum over sampled tail cols of softplus(logits) * not_cand * 980
    loss    = mean over batch of (term1 + term2 + term3)

Only ~2110 of the 100000 columns are ever read per row.  Sharding is
data-parallel over batch (64 rows per core, 8 cores).  Each core receives
its logits shard TRANSPOSED ([C, 64], so a class column is a contiguous
64-float run) and reads just what it needs out of DRAM:
  - head block  [2000, 64]: one contiguous 512 KB DMA -> [125, 1024] tile
  - sampled cols: ONE indirect DMA, offset per partition (100 x 64-run)
  - candidates:  5 indirect DMAs of 128 single-element gathers
softplus(x) = Ln(1*exp(x) + 1) on the Scalar engine (both funcs in the
natural_log_exp_and_others table set - one table load); row/partition sums
ride the activation/scalar_tensor_tensor accumulators.  Each core outputs
a handful of partial sums; the host mean-reduces them (per sharding hint).
"""

import sys

if "/opt/trn_rl_repo" not in sys.path:
    sys.path.insert(0, "/opt/trn_rl_repo")

import numpy as np

B, C, HEAD, K, S = 512, 100000, 2000, 10, 100
NCORES = 8
RB = B // NCORES            # rows per core
TAIL = C - HEAD
SCALE3 = float(TAIL) / S    # 980.0
HP = 125                    # head partitions: 2000*64 = 125 * 1024
HF = HEAD * RB // HP        # 1024
KD = 5                      # candidate gathers: 5 DMAs x 128 offsets = 640

_BUILT = None


def _legalize_waits(nc):
    """Split >cap sync waits onto preceding NoOps (walrus codegen accepts at
    most 1 wait per instruction, 2 on EventSemaphore; Tile attaches more)."""
    from concourse import mybir

    cnt = 0
    for bfn in nc.m.functions:
        for blk in bfn.blocks:
            out = []
            changed = False
            for inst in blk.instructions:
                si = inst.sync_info
                waits = list(si.on_wait) if si is not None and si.on_wait else []
                cap = 2 if isinstance(inst, mybir.InstEventSemaphore) else 1
                if len(waits) > cap:
                    changed = True
                    keep = waits[-cap:]
                    for w in waits[:-cap]:
                        cnt += 1
                        out.append(mybir.InstNoOp(
                            name=f"WSPLIT-{cnt}",
                            engine=inst.engine,
                            sync_info=mybir.SyncInfo(on_wait=[w], on_update=[]),
                            bass_nofuse=True,
                        ))
                    inst.sync_info = mybir.SyncInfo(
                        on_wait=keep,
                        on_update=list(si.on_update) if si.on_update else [],
                    )
                out.append(inst)
            if changed:
                blk.instructions = out
    return nc


def _build():
    from concourse import bass, mybir, tile

    f32 = mybir.dt.float32
    i32 = mybir.dt.int32
    F = mybir.ActivationFunctionType
    A = mybir.AluOpType

    nc = bass.Bass()
    lgT = nc.declare_dram_parameter("logits_t", [C, RB], f32, isOutput=False)
    samp_off = nc.declare_dram_parameter("samp_off", [S, 1], i32, isOutput=False)
    cand_off = nc.declare_dram_parameter("cand_off", [2 * RB, KD], i32, isOutput=False)
    w1p = nc.declare_dram_parameter("w1p", [2 * RB, KD], f32, isOutput=False)
    w2p = nc.declare_dram_parameter("w2p", [2 * RB, KD], f32, isOutput=False)
    m3t = nc.declare_dram_parameter("m3t", [S, RB], f32, isOutput=False)
    out = nc.dram_tensor("out", [2 * RB, 4], f32, kind="ExternalOutput")

    with tile.TileContext(nc) as tc:
        with tc.tile_pool(name="p", bufs=1) as pool:
            # result accumulators: col0=head, col1=term3, col2=c2, col3=t1
            res_t = pool.tile([2 * RB, 4], f32)
            nc.vector.memset(res_t[:], 0.0)

            # --- head block: contiguous [2000, 64] -> [125, 1024] --------
            head_t = pool.tile([HP, HF], f32)
            nc.sync.dma_start(
                out=head_t[:],
                in_=lgT[0:HEAD, :].rearrange("(p a) r -> p (a r)", p=HP),
            )
            heade = pool.tile([HP, HF], f32)
            nc.scalar.activation(heade[:], head_t[:], F.Exp)
            headsp = pool.tile([HP, HF], f32)
            nc.scalar.activation(
                headsp[:], heade[:], F.Ln, bias=1.0,
                accum_out=res_t[0:HP, 0:1],
            )

            # --- sampled tail columns: one indirect gather ---------------
            so_t = pool.tile([S, 1], i32)
            nc.sync.dma_start(out=so_t[:], in_=samp_off[:])
            samp_t = pool.tile([S, RB], f32)
            nc.gpsimd.indirect_dma_start(
                out=samp_t[:], out_offset=None, in_=lgT[:],
                in_offset=bass.IndirectOffsetOnAxis(ap=so_t[:], axis=0),
            )
            m3_t = pool.tile([S, RB], f32)
            nc.sync.dma_start(out=m3_t[:], in_=m3t[:])
            sexp = pool.tile([S, RB], f32)
            nc.scalar.activation(sexp[:], samp_t[:], F.Exp)
            ssp = pool.tile([S, RB], f32)
            nc.scalar.activation(ssp[:], sexp[:], F.Ln, bias=1.0)
            t3p = pool.tile([S, RB], f32)
            nc.vector.scalar_tensor_tensor(
                out=t3p[:], in0=ssp[:], scalar=1.0, in1=m3_t[:],
                op0=A.mult, op1=A.mult, accum_out=res_t[0:S, 1:2],
            )

            # --- candidates: 5 x 128 single-element gathers --------------
            co_t = pool.tile([2 * RB, KD], i32)
            nc.sync.dma_start(out=co_t[:], in_=cand_off[:])
            w1_t = pool.tile([2 * RB, KD], f32)
            nc.sync.dma_start(out=w1_t[:], in_=w1p[:])
            w2_t = pool.tile([2 * RB, KD], f32)
            nc.sync.dma_start(out=w2_t[:], in_=w2p[:])
            cand_t = pool.tile([2 * RB, KD], f32)
            for i in range(KD):
                nc.gpsimd.indirect_dma_start(
                    out=cand_t[:, i:i + 1], out_offset=None, in_=lgT[:],
                    in_offset=bass.IndirectOffsetOnAxis(
                        ap=co_t[:, i:i + 1], axis=1,
                    ),
                )
            cexp = pool.tile([2 * RB, KD], f32)
            nc.scalar.activation(cexp[:], cand_t[:], F.Exp)
            csp = pool.tile([2 * RB, KD], f32)
            nc.scalar.activation(csp[:], cexp[:], F.Ln, bias=1.0)
            c2p = pool.tile([2 * RB, KD], f32)
            nc.vector.scalar_tensor_tensor(
                out=c2p[:], in0=csp[:], scalar=1.0, in1=w2_t[:],
                op0=A.mult, op1=A.mult, accum_out=res_t[:, 2:3],
            )

            # --- term1: avg over candidates, pair-add across halves ------
            avgp = pool.tile([2 * RB, KD], f32)
            avg_acc = pool.tile([2 * RB, 1], f32)
            nc.vector.scalar_tensor_tensor(
                out=avgp[:], in0=cand_t[:], scalar=1.0, in1=w1_t[:],
                op0=A.mult, op1=A.mult, accum_out=avg_acc[:],
            )
            shift_t = pool.tile([RB, 1], f32)
            nc.sync.dma_start(out=shift_t[:], in_=avg_acc[RB:2 * RB, :])
            avg2 = pool.tile([RB, 1], f32)
            nc.vector.tensor_tensor(
                out=avg2[:], in0=avg_acc[0:RB, :], in1=shift_t[:], op=A.add,
            )
            t1e = pool.tile([RB, 1], f32)
            nc.scalar.activation(t1e[:], avg2[:], F.Exp, scale=-1.0)
            nc.scalar.activation(
                res_t[0:RB, 3:4], t1e[:], F.Ln, bias=1.0,
            )

            nc.sync.dma_start(out=out[:], in_=res_t[:])

    _legalize_waits(nc)
    return nc


def _get_built():
    global _BUILT
    if _BUILT is None:
        _BUILT = _build()
    return _BUILT


def _host_prep(candidates, sampled_idx):
    """Index-only host prep: dedup/weight masks + gather offsets."""
    cand = np.asarray(candidates)
    samp = np.asarray(sampled_idx).reshape(-1)
    valid = cand >= 0                                        # [B, K]

    # first-occurrence mask over valid candidates (set semantics)
    W = np.zeros((B, K), np.float32)
    for k in range(K):
        dup = np.zeros(B, bool)
        for j in range(k):
            dup |= valid[:, j] & (cand[:, j] == cand[:, k])
        W[:, k] = (valid[:, k] & ~dup).astype(np.float32)

    ycard = np.maximum(W.sum(axis=1), 1.0).astype(np.float32)   # [B]
    w1 = (W / ycard[:, None]).astype(np.float32)                # [B, K]
    w2 = (W * (cand < HEAD)).astype(np.float32)                 # [B, K]

    g = (HEAD + samp).astype(np.int64)                          # [S]
    is_cand = (valid[:, :, None] & (cand[:, :, None] == g[None, None, :])).any(
        axis=1
    )                                                           # [B, S]
    m3 = (SCALE3 * (~is_cand)).astype(np.float32)               # [B, S]

    cand_pos = np.where(valid, cand, 0).astype(np.int64)        # [B, K]
    samp_off = g.astype(np.int32).reshape(S, 1)                 # [S, 1]
    return w1, w2, m3, cand_pos, samp_off


def _pack_cand(arr_rk):
    """[RB, K] row-major per-core array -> [128, KD] packed layout where
    DMA column i, partition p holds (r = p % RB, k = 2*i + p // RB)."""
    out = np.empty((2 * RB, KD), arr_rk.dtype)
    p = np.arange(2 * RB)
    r = p % RB
    for i in range(KD):
        k = 2 * i + p // RB
        out[:, i] = arr_rk[r, k]
    return out


def kernel(logits, candidates, sampled_idx):
    from concourse.bass_utils import run_bass_kernel_spmd

    logits = np.asarray(logits, dtype=np.float32)
    w1, w2, m3, cand_pos, samp_off = _host_prep(candidates, sampled_idx)

    in_maps = []
    p = np.arange(2 * RB)
    r_of_p = p % RB
    for i in range(NCORES):
        sl = slice(i * RB, (i + 1) * RB)
        # flat offsets into the transposed shard: c*RB + r
        coff = _pack_cand(cand_pos[sl]).astype(np.int64) * RB + r_of_p[:, None]
        in_maps.append({
            "logits_t": np.ascontiguousarray(logits[sl].T),
            "samp_off": samp_off,
            "cand_off": coff.astype(np.int32),
            "w1p": _pack_cand(w1[sl]),
            "w2p": _pack_cand(w2[sl]),
            "m3t": np.ascontiguousarray(m3[sl].T),
        })

    nc = _get_built()
    res = run_bass_kernel_spmd(nc, in_maps, core_ids=list(range(NCORES)))
    total = 0.0
    for i in range(NCORES):
        o = res.results[i]["out"].astype(np.float64)
        total += o[:, 0].sum() + o[:, 1].sum() - o[:, 2].sum() + o[:, 3].sum()
    return np.float32(total / B)


# revision 12
# speedup vs baseline: 1.5459x; 1.5459x over previous
"""Trainium2 Bass kernel for AdaptiveCLPLLoss.

Reference computation (B=512, C=100000, HEAD=2000, K=10, S=100):
    logits  [B, C] f32, candidates [B, K] i64, sampled_idx [S] i64
    y_mask  = binarized scatter of valid candidates            [B, C]
    term1   = softplus(-avg_cand)     avg over distinct candidate logits
    term2   = sum over head cols of softplus(logits) * (1 - y_mask)
    term3   = sum over sampled tail cols of softplus(logits) * not_cand * 980
    loss    = mean over batch of (term1 + term2 + term3)

Only ~2110 of the 100000 columns are ever read per row.  Sharding is
data-parallel over batch (64 rows per core, 8 cores).  Each core receives
its logits shard TRANSPOSED ([C, 64], so a class column is a contiguous
64-float run) and reads just what it needs out of DRAM:
  - head block  [2000, 64]: one contiguous 512 KB DMA -> [125, 1024] tile
  - sampled cols: ONE indirect DMA, offset per partition (100 x 64-run)
  - candidates:  5 indirect DMAs of 128 single-element gathers
softplus(x) = Ln(1*exp(x) + 1) on the Scalar engine (both funcs in the
natural_log_exp_and_others table set - one table load); row/partition sums
ride the activation/scalar_tensor_tensor accumulators.  Each core outputs
a handful of partial sums; the host mean-reduces them (per sharding hint).
"""

import sys

if "/opt/trn_rl_repo" not in sys.path:
    sys.path.insert(0, "/opt/trn_rl_repo")

import numpy as np

B, C, HEAD, K, S = 512, 100000, 2000, 10, 100
NCORES = 8
RB = B // NCORES            # rows per core
TAIL = C - HEAD
SCALE3 = float(TAIL) / S    # 980.0
HP = 125                    # head partitions: 2000*64 = 125 * 1024
HF = HEAD * RB // HP        # 1024
KD = 5                      # candidate gathers: 5 DMAs x 128 offsets = 640

_BUILT = None


def _legalize_waits(nc):
    """Split >cap sync waits onto preceding NoOps (walrus codegen accepts at
    most 1 wait per instruction, 2 on EventSemaphore; Tile attaches more)."""
    from concourse import mybir

    cnt = 0
    for bfn in nc.m.functions:
        for blk in bfn.blocks:
            out = []
            changed = False
            for inst in blk.instructions:
                si = inst.sync_info
                waits = list(si.on_wait) if si is not None and si.on_wait else []
                cap = 2 if isinstance(inst, mybir.InstEventSemaphore) else 1
                if len(waits) > cap:
                    changed = True
                    keep = waits[-cap:]
                    for w in waits[:-cap]:
                        cnt += 1
                        out.append(mybir.InstNoOp(
                            name=f"WSPLIT-{cnt}",
                            engine=inst.engine,
                            sync_info=mybir.SyncInfo(on_wait=[w], on_update=[]),
                            bass_nofuse=True,
                        ))
                    inst.sync_info = mybir.SyncInfo(
                        on_wait=keep,
                        on_update=list(si.on_update) if si.on_update else [],
                    )
                out.append(inst)
            if changed:
                blk.instructions = out
    return nc


def _build():
    from concourse import bass, mybir, tile

    f32 = mybir.dt.float32
    i32 = mybir.dt.int32
    F = mybir.ActivationFunctionType
    A = mybir.AluOpType

    nc = bass.Bass()
    lgT = nc.declare_dram_parameter("logits_t", [C, RB], f32, isOutput=False)
    # aux layout per partition p: [0:64]=m3t row (p<S), [64:69]=w1p,
    # [69:74]=w2p, [74:79]=cand_off (i32 bits), [79]=samp_off (i32, p<S)
    aux = nc.declare_dram_parameter("aux", [2 * RB, 80], f32, isOutput=False)
    out = nc.dram_tensor("out", [2 * RB, 4], f32, kind="ExternalOutput")

    with tile.TileContext(nc) as tc:
        with tc.tile_pool(name="p", bufs=1) as pool:
            # result accumulators: col0=head, col1=term3, col2=c2, col3=t1
            res_t = pool.tile([2 * RB, 4], f32)
            nc.vector.memset(res_t[:], 0.0)

            # one aux DMA (vector-issued: parallel with head DMA on sync)
            aux_t = pool.tile([2 * RB, 80], f32)
            nc.scalar.dma_start(out=aux_t[:], in_=aux[:])
            m3_s = aux_t[0:S, 0:RB]
            w1_s = aux_t[:, 64:64 + KD]
            w2_s = aux_t[:, 69:69 + KD]
            co_s = aux_t[:, 74:74 + KD].bitcast(i32)
            so_s = aux_t[0:S, 79:80].bitcast(i32)

            # --- head block: contiguous [2000, 64] -> [125, 1024] --------
            head_t = pool.tile([HP, HF], f32)
            nc.sync.dma_start(
                out=head_t[:],
                in_=lgT[0:HEAD, :].rearrange("(p a) r -> p (a r)", p=HP),
            )
            heade = pool.tile([HP, HF], f32)
            nc.scalar.activation(heade[:], head_t[:], F.Exp)
            headsp = pool.tile([HP, HF], f32)
            nc.scalar.activation(
                headsp[:], heade[:], F.Ln, bias=1.0,
                accum_out=res_t[0:HP, 0:1],
            )

            # --- candidates first: 5 x 128 single-element gathers --------
            cand_t = pool.tile([2 * RB, KD], f32)
            for i in range(KD):
                nc.gpsimd.indirect_dma_start(
                    out=cand_t[:, i:i + 1], out_offset=None, in_=lgT[:],
                    in_offset=bass.IndirectOffsetOnAxis(
                        ap=co_s[:, i:i + 1], axis=1,
                    ),
                )
            cexp = pool.tile([2 * RB, KD], f32)
            nc.scalar.activation(cexp[:], cand_t[:], F.Exp)
            csp = pool.tile([2 * RB, KD], f32)
            nc.scalar.activation(csp[:], cexp[:], F.Ln, bias=1.0)
            c2p = pool.tile([2 * RB, KD], f32)
            nc.vector.scalar_tensor_tensor(
                out=c2p[:], in0=csp[:], scalar=1.0, in1=w2_s,
                op0=A.mult, op1=A.mult, accum_out=res_t[:, 2:3],
            )

            # --- term1: avg over candidates, pair-add across halves ------
            avgp = pool.tile([2 * RB, KD], f32)
            avg_acc = pool.tile([2 * RB, 1], f32)
            nc.vector.scalar_tensor_tensor(
                out=avgp[:], in0=cand_t[:], scalar=1.0, in1=w1_s,
                op0=A.mult, op1=A.mult, accum_out=avg_acc[:],
            )
            shift_t = pool.tile([RB, 1], f32)
            nc.sync.dma_start(out=shift_t[:], in_=avg_acc[RB:2 * RB, :])
            avg2 = pool.tile([RB, 1], f32)
            nc.vector.tensor_tensor(
                out=avg2[:], in0=avg_acc[0:RB, :], in1=shift_t[:], op=A.add,
            )
            t1e = pool.tile([RB, 1], f32)
            nc.scalar.activation(t1e[:], avg2[:], F.Exp, scale=-1.0)
            nc.scalar.activation(
                res_t[0:RB, 3:4], t1e[:], F.Ln, bias=1.0,
            )

            # --- sampled tail columns: one indirect gather (last) --------
            samp_t = pool.tile([S, RB], f32)
            nc.gpsimd.indirect_dma_start(
                out=samp_t[:], out_offset=None, in_=lgT[:],
                in_offset=bass.IndirectOffsetOnAxis(ap=so_s, axis=0),
            )
            sexp = pool.tile([S, RB], f32)
            nc.scalar.activation(sexp[:], samp_t[:], F.Exp)
            ssp = pool.tile([S, RB], f32)
            nc.scalar.activation(ssp[:], sexp[:], F.Ln, bias=1.0)
            t3p = pool.tile([S, RB], f32)
            nc.vector.scalar_tensor_tensor(
                out=t3p[:], in0=ssp[:], scalar=1.0, in1=m3_s,
                op0=A.mult, op1=A.mult, accum_out=res_t[0:S, 1:2],
            )

            nc.sync.dma_start(out=out[:], in_=res_t[:])

    _legalize_waits(nc)
    return nc


def _get_built():
    global _BUILT
    if _BUILT is None:
        _BUILT = _build()
    return _BUILT


def _host_prep(candidates, sampled_idx):
    """Index-only host prep: dedup/weight masks + gather offsets."""
    cand = np.asarray(candidates)
    samp = np.asarray(sampled_idx).reshape(-1)
    valid = cand >= 0                                        # [B, K]

    # first-occurrence mask over valid candidates (set semantics)
    W = np.zeros((B, K), np.float32)
    for k in range(K):
        dup = np.zeros(B, bool)
        for j in range(k):
            dup |= valid[:, j] & (cand[:, j] == cand[:, k])
        W[:, k] = (valid[:, k] & ~dup).astype(np.float32)

    ycard = np.maximum(W.sum(axis=1), 1.0).astype(np.float32)   # [B]
    w1 = (W / ycard[:, None]).astype(np.float32)                # [B, K]
    w2 = (W * (cand < HEAD)).astype(np.float32)                 # [B, K]

    g = (HEAD + samp).astype(np.int64)                          # [S]
    is_cand = (valid[:, :, None] & (cand[:, :, None] == g[None, None, :])).any(
        axis=1
    )                                                           # [B, S]
    m3 = (SCALE3 * (~is_cand)).astype(np.float32)               # [B, S]

    cand_pos = np.where(valid, cand, 0).astype(np.int64)        # [B, K]
    samp_off = g.astype(np.int32).reshape(S, 1)                 # [S, 1]
    return w1, w2, m3, cand_pos, samp_off


def _pack_cand(arr_rk):
    """[RB, K] row-major per-core array -> [128, KD] packed layout where
    DMA column i, partition p holds (r = p % RB, k = 2*i + p // RB)."""
    out = np.empty((2 * RB, KD), arr_rk.dtype)
    p = np.arange(2 * RB)
    r = p % RB
    for i in range(KD):
        k = 2 * i + p // RB
        out[:, i] = arr_rk[r, k]
    return out


def kernel(logits, candidates, sampled_idx):
    from concourse.bass_utils import run_bass_kernel_spmd

    logits = np.asarray(logits, dtype=np.float32)
    w1, w2, m3, cand_pos, samp_off = _host_prep(candidates, sampled_idx)

    in_maps = []
    p = np.arange(2 * RB)
    r_of_p = p % RB
    for i in range(NCORES):
        sl = slice(i * RB, (i + 1) * RB)
        # flat offsets into the transposed shard: c*RB + r
        coff = _pack_cand(cand_pos[sl]).astype(np.int64) * RB + r_of_p[:, None]
        a = np.zeros((2 * RB, 80), np.float32)
        a[0:S, 0:RB] = m3[sl].T
        a[:, 64:64 + KD] = _pack_cand(w1[sl])
        a[:, 69:69 + KD] = _pack_cand(w2[sl])
        a[:, 74:74 + KD].view(np.int32)[:] = coff.astype(np.int32)
        a[0:S, 79:80].view(np.int32)[:] = samp_off
        in_maps.append({
            "logits_t": np.ascontiguousarray(logits[sl].T),
            "aux": a,
        })

    nc = _get_built()
    res = run_bass_kernel_spmd(nc, in_maps, core_ids=list(range(NCORES)))
    total = 0.0
    for i in range(NCORES):
        o = res.results[i]["out"].astype(np.float64)
        total += o[:, 0].sum() + o[:, 1].sum() - o[:, 2].sum() + o[:, 3].sum()
    return np.float32(total / B)
